# revision 1
# baseline (speedup 1.0000x reference)
"""MinibatchDiscrimination Trainium2 kernel — symmetric pairwise split.

Reference computation:
    M = x @ T.reshape(512, 128*16)           -> [256, 128, 16]
    norm[a,b,o] = sum_k |M[a,o,k] - M[b,o,k]|
    o_b[b,o]    = sum_a exp(-norm[a,b,o])
    out = concat([x, o_b], axis=1)           -> [256, 640]

Distribution: the 256x256 pairwise matrix is covered once per unordered
pair. b is split into 16 columns of 16; K16 edges are oriented (gap 1..7
toward the later column, antipodal I->I+8) so columns 0..7 need 16
a-octets and columns 8..15 need 18. Core d owns columns d and d+8 = 34
octet-tiles per core (vs 64 for the row-parallel baseline). The per-core
a-octet selection is pure input data: the host feeds a permuted
xT with 272 a-slots (duplicates allowed), so all 8 cores run the same
program (SPMD) on different slot->a mappings.

Engine assignment (loop steady state, per iteration):
  - DVE: 17 mega tensor_tensor max ops (2 octets per op, 3-free-dim APs;
    max-decomposition |u-v| = 2*max(u,v)-u-v). This is the bottleneck
    (~0.5 ns/elem 2x mode wall).
  - PE: 136 k-sum matmuls (blk2 block-diag selectors, entries 2.0,
    accumulating 2*sum_k max into PSUM), 36 Sab-correction matmuls
    (3-term bf16 hi/mid/lo split of Sab streamed through -I; diagonal
    error <= ~1e-4), 12 P_col ones-matmuls (emitted one group late so
    they never wait on fresh exps).
  - ScalarE: 12 exp ops (PSUM -> et bf16).
  - Pool: P_row b-folds 16->8->4 (TT-adds; host finishes the sum).
Deep buffering is critical on HW (p-states + in-order queues): at-pool 6
bufs, 6 norm PSUM banks. The host sums P_col + credited P_row rows
(self-octet rows excluded, +analytic handling baked into coverage).
"""

import numpy as np
import ml_dtypes

import concourse.bass as bass
import concourse.tile as tile
from concourse import bacc, mybir
from concourse.bass_utils import run_bass_kernel_spmd

BF16 = ml_dtypes.bfloat16
B = 256          # batch
IN_F = 512       # in_features
OUT_F = 128      # out_features (o)
KD = 16          # kernel_dims (k)
NCORES = 8
NO = KD * OUT_F   # 2048, (k,o) free size
NCOL = 16         # b-columns
CB = B // NCOL    # 16 b per column
NSLOT = 272       # per-core a-slots: col A 128 + col B 144
NJS = 4           # 512-wide psum chunks per column (16b x 128o = 2048)
JW = CB * OUT_F   # 2048 free width per column

# (column, partition-group) loop structure: (m-octets, slot base, col, parts)
GROUPS = [
    (16, 0, 0, 128),    # col A (owner core d): slots 0..127
    (16, 128, 1, 128),  # col B h1: slots 128..255
    (2, 256, 1, 16),    # col B h2: slots 256..271
]

MEG = 2           # octets per mega max op
AT_BUFS = 6       # at4 pool depth
NP_BUFS = 6       # norm psum banks
MP_BUFS = 1       # prologue psum banks

AluOp = mybir.AluOpType
Act = mybir.ActivationFunctionType
f32 = mybir.dt.float32
bf16 = mybir.dt.bfloat16


def _build_kernel(loop_reps=None, variant="full"):
    nc = bacc.Bacc("TRN2", target_bir_lowering=False, debug=False)
    xT = nc.dram_tensor("xT", [IN_F, NSLOT], bf16, kind="ExternalInput")
    t2 = nc.dram_tensor("t2", [IN_F, NO], bf16, kind="ExternalInput")
    xTb = nc.dram_tensor("xTb", [IN_F, 2 * CB], bf16, kind="ExternalInput")
    blk = nc.dram_tensor("blk", [128, 16 * 128], bf16, kind="ExternalInput")
    negI = nc.dram_tensor("negI", [128, 128], bf16, kind="ExternalInput")
    obc = nc.dram_tensor("obc", [8, 512], f32, kind="ExternalOutput")
    obr = nc.dram_tensor("obr", [128, 3 * 512], bf16, kind="ExternalOutput")

    with tile.TileContext(nc) as tc:
        _body(tc, xT[:], t2[:], xTb[:], blk[:], negI[:], obc[:], obr[:],
              loop_reps, variant)
    nc.compile()
    return nc


def _body(tc, xT, t2, xTb, blk, negI, obc, obr, loop_reps=None, variant="full"):
    nc = tc.nc
    from contextlib import ExitStack

    with ExitStack() as ctx:
        singles = ctx.enter_context(tc.tile_pool(name="singles", bufs=1))
        mpsum = ctx.enter_context(tc.tile_pool(name="mpsum", bufs=MP_BUFS, space="PSUM"))
        npsum = ctx.enter_context(tc.tile_pool(name="npsum", bufs=NP_BUFS, space="PSUM"))
        obpsum = ctx.enter_context(tc.tile_pool(name="obpsum", bufs=1, space="PSUM"))
        apool = ctx.enter_context(tc.tile_pool(name="apool", bufs=AT_BUFS))
        epool = ctx.enter_context(tc.tile_pool(name="epool", bufs=3))
        opool = ctx.enter_context(tc.tile_pool(name="opool", bufs=2))

        # ---- load inputs ----
        xT_s = singles.tile([128, 4, NSLOT], bf16)
        t2_s = singles.tile([128, 4, NO], bf16)
        xTb_s = singles.tile([128, 4, 2 * CB], bf16)
        blk_s = singles.tile([128, 16 * 128], bf16)
        negI_s = singles.tile([128, 128], bf16)
        for cc in range(4):
            sl = slice(cc * 128, (cc + 1) * 128)
            nc.sync.dma_start(out=xT_s[:, cc, :], in_=xT[sl, :])
            nc.sync.dma_start(out=t2_s[:, cc, :], in_=t2[sl, :])
            nc.sync.dma_start(out=xTb_s[:, cc, :], in_=xTb[sl, :])
        nc.sync.dma_start(out=blk_s[:], in_=blk[:, :])
        nc.sync.dma_start(out=negI_s[:], in_=negI[:, :])

        # ones-column selector: onepad[:, 128-r:256-r] has column r all-ones
        onepad = singles.tile([128, 256], bf16)
        nc.vector.memset(onepad[:], 0.0)
        nc.vector.memset(onepad[:, 128:129], 1.0)

        # ---- M2[slot, (k,o)] = xT.T @ t2, 3 partition-groups ----
        M2 = singles.tile([128, 3, NO], bf16)
        for hg in range(3):
            p = 128 if hg < 2 else 16
            for jc4 in range(4):
                pm = mpsum.tile([128, 512], f32)
                for cc in range(4):
                    nc.tensor.matmul(
                        pm[:p, :],
                        xT_s[:, cc, hg * 128:hg * 128 + p],
                        t2_s[:, cc, jc4 * 512:(jc4 + 1) * 512],
                        start=(cc == 0),
                        stop=(cc == 3),
                    )
                nc.scalar.copy(M2[:p, hg, jc4 * 512:(jc4 + 1) * 512], pm[:p, :])

        # ---- M2b[bl, (k,o)] for this core's 32 b-rows (2 columns) ----
        M2b = singles.tile([2 * CB, NO], bf16)
        for jc4 in range(4):
            pm = mpsum.tile([2 * CB, 512], f32)
            for cc in range(4):
                nc.tensor.matmul(
                    pm[:],
                    xTb_s[:, cc, :],
                    t2_s[:, cc, jc4 * 512:(jc4 + 1) * 512],
                    start=(cc == 0),
                    stop=(cc == 3),
                )
            nc.scalar.copy(M2b[:, jc4 * 512:(jc4 + 1) * 512], pm[:])

        # ---- M3[(a8,k), (s,o)]: k-on-partition octet sheets, 34 sheets ----
        M3 = singles.tile([128, 34 * OUT_F], bf16)
        for s in range(34):
            hg, m = s // 16, s % 16
            nc.gpsimd.dma_start(
                out=M3[:, s * OUT_F:(s + 1) * OUT_F],
                in_=M2[m * 8:(m + 1) * 8, hg, :],
            )

        # ---- MBrep[(a8,k), (bl,o)]: both columns' b in k-on-partition ----
        MBrep = singles.tile([128, 2 * CB * OUT_F], bf16)
        for bl in range(2 * CB):
            dst = MBrep[0:KD, bl * OUT_F:(bl + 1) * OUT_F]
            src = M2b[bl:bl + 1, :].rearrange("p (k o) -> p k o", k=KD)
            nc.gpsimd.dma_start(out=dst, in_=src)
        for r in (16, 32, 64):
            nc.gpsimd.dma_start(out=MBrep[r:2 * r, :], in_=MBrep[0:r, :])

        # ---- S sums (max-decomposition corrections) ----
        blk2_s = singles.tile([128, 16 * 128], bf16)
        nc.vector.tensor_scalar_mul(blk2_s[:], blk_s[:], 2.0)

        S_ah = singles.tile([128, 3, OUT_F], f32)
        for hg in range(3):
            nm, p = (16, 128) if hg < 2 else (2, 16)
            psa = mpsum.tile([128, OUT_F], f32, name=f"psa_{hg}", tag="pm")
            for m in range(nm):
                nc.tensor.matmul(
                    psa[:, :],
                    blk_s[:, m * 128:(m + 1) * 128],
                    M3[:, (hg * 16 + m) * OUT_F:(hg * 16 + m + 1) * OUT_F],
                    start=(m == 0),
                    stop=(m == nm - 1),
                )
            nc.vector.tensor_copy(S_ah[:p, hg, :], psa[:p, :])

        # blkrep[c, p] = 1 iff p % 8 == a8(c)
        blkrep_f = singles.tile([128, 128], f32)
        bview = bass.AP(
            tensor=blk_s[:].tensor,
            offset=blk_s[:].offset,
            ap=[list(blk_s[:].ap[0]), [1, 128], [128, 16]],
        )
        nc.vector.tensor_reduce(blkrep_f[:], bview, axis=mybir.AxisListType.X,
                                op=AluOp.add)
        blkrep = singles.tile([128, 128], bf16)
        nc.vector.tensor_copy(blkrep[:], blkrep_f[:])

        SBp = singles.tile([128, 2 * CB * OUT_F], f32)
        for ch in range(8):
            psb = mpsum.tile([128, 512], f32, name=f"psb_{ch}", tag="pm")
            nc.tensor.matmul(
                psb[:],
                blkrep[:],
                MBrep[:, ch * 512:(ch + 1) * 512],
                start=True,
                stop=True,
            )
            nc.scalar.copy(SBp[:, ch * 512:(ch + 1) * 512], psb[:])

        # Sab = S_a (bcast over b) + S_b per group, then 3-term bf16 split:
        # the PE subtracts hi/mid/lo from the norm PSUM chain via negI
        # matmuls; diagonal error <= ~1e-4.
        S3 = singles.tile([128, 3, 3, JW], bf16)
        Sab = singles.tile([128, JW], f32)
        Stmp = singles.tile([128, JW], f32)
        for gi, (nm, sbase, col, p) in enumerate(GROUPS):
            base = S_ah[:p, gi, :]
            in0 = bass.AP(
                tensor=base.tensor,
                offset=base.offset,
                ap=[list(base.ap[0]), [0, CB], list(base.ap[1])],
            )
            in1 = SBp[:p, col * JW:(col + 1) * JW].rearrange(
                "p (b o) -> p b o", b=CB
            )
            out = Sab[:p, :].rearrange("p (b o) -> p b o", b=CB)
            nc.vector.tensor_tensor(out, in0, in1, AluOp.add)
            hi = S3[:p, gi, 0, :]
            mid = S3[:p, gi, 1, :]
            lo = S3[:p, gi, 2, :]
            nc.vector.tensor_copy(hi, Sab[:p, :])
            nc.vector.tensor_tensor(Stmp[:p, :], Sab[:p, :], hi,
                                    AluOp.subtract)
            nc.vector.tensor_copy(mid, Stmp[:p, :])
            nc.vector.tensor_tensor(lo, Stmp[:p, :], mid, AluOp.subtract)

        # ---- main pairwise loop ----
        ob_ps = obpsum.tile([128, 512], f32)

        def _main():
            _pairwise(tc, apool, epool, opool, npsum, M3, MBrep, blk2_s, S3,
                      negI_s, onepad, ob_ps, obr, variant)
            if variant == "full":
                ob_sb = opool.tile([8, 512], f32, name="ob_sb")
                nc.scalar.copy(ob_sb[:], ob_ps[:8, :])
                nc.sync.dma_start(out=obc, in_=ob_sb[:])

        if isinstance(loop_reps, str) and loop_reps.startswith("unroll"):
            for _ in range(int(loop_reps[6:])):
                _main()
        elif loop_reps is None or (
                not isinstance(loop_reps, tuple) and loop_reps <= 1):
            _main()
        else:
            if isinstance(loop_reps, tuple):
                n_iter, unroll = loop_reps
            else:
                n_iter, unroll = loop_reps, 1
            with tc.For_i(0, n_iter, 1, hint_engines=(
                    mybir.EngineType.PE, mybir.EngineType.DVE,
                    mybir.EngineType.Activation, mybir.EngineType.Pool)):
                for _ in range(unroll):
                    _main()


def _pairwise(tc, apool, epool, opool, npsum, M3, MBrep, blk2_s, S3,
              negI_s, onepad, ob_ps, obr, variant="full"):
    nc = tc.nc
    obr_sb = opool.tile([128, 3 * 512], bf16, name="obr_sb")
    ets = []
    nob = [0]

    def _emit_obs_folds(gi):
        # P_col ones-matmuls (PE) + P_row folds (Pool) for group gi; emitted
        # one group late so the PE/Pool never wait on fresh exps
        nm, sbase, col, p = GROUPS[gi]
        et = ets[gi]
        for js in range(NJS):
            r = col * NJS + js
            nob[0] += 1
            nc.tensor.matmul(
                ob_ps[:, :],
                onepad[:p, 128 - r:256 - r],
                et[:p, js * 512:(js + 1) * 512],
                start=(nob[0] == 1),
                stop=(nob[0] == 12),
            )
        f1 = epool.tile([128, 1024], bf16, name="f1")
        nc.gpsimd.tensor_tensor(
            f1[:p, :], et[:p, 0:1024], et[:p, 1024:2048], AluOp.add
        )
        nc.gpsimd.tensor_tensor(
            obr_sb[:p, gi * 512:(gi + 1) * 512],
            f1[:p, 0:512], f1[:p, 512:1024], AluOp.add,
        )

    for gi, (nm, sbase, col, p) in enumerate(GROUPS):
        norm_ps = [
            npsum.tile([128, 512], f32, tag="norm", name=f"norm_{gi}_{js}")
            for js in range(NJS)
        ]
        # mega max: MEG octets per DVE op (amortizes per-op overhead)
        for mg in range((nm + MEG - 1) // MEG):
            moct = min(MEG, nm - mg * MEG)
            s0 = gi * 16 + mg * MEG
            base = M3[:, s0 * OUT_F:(s0 + moct) * OUT_F]
            in0 = bass.AP(
                tensor=base.tensor,
                offset=base.offset,
                ap=[list(base.ap[0]), [OUT_F, moct], [0, CB], [1, OUT_F]],
            )
            mb = MBrep[:, col * JW:(col + 1) * JW]
            in1 = bass.AP(
                tensor=mb.tensor,
                offset=mb.offset,
                ap=[list(mb.ap[0]), [0, moct], [OUT_F, CB], [1, OUT_F]],
            )
            at = apool.tile([128, MEG * JW], bf16, name="at4")
            atv = at[:, 0:moct * JW].rearrange(
                "p (g b o) -> p g b o", g=moct, b=CB
            )
            nc.vector.tensor_tensor(atv, in0, in1, AluOp.max)
            for g in range(moct):
                m = mg * MEG + g
                for js in range(NJS):
                    nc.tensor.matmul(
                        norm_ps[js][:, :],
                        blk2_s[:, m * 128:(m + 1) * 128],
                        at[:, g * JW + js * 512:g * JW + (js + 1) * 512],
                        start=(m == 0),
                        stop=False,
                    )
        et = epool.tile([128, JW], bf16, name="et")
        ets.append(et)
        for js in range(NJS):
            # subtract Sab on the PE: hi/mid/lo bf16 terms via -I matmuls
            for t in range(3):
                nc.tensor.matmul(
                    norm_ps[js][:p, :],
                    negI_s[:p, :p],
                    S3[:p, gi, t, js * 512:(js + 1) * 512],
                    start=False,
                    stop=(t == 2),
                )
            if variant != "noex":
                nc.scalar.activation(
                    et[:p, js * 512:(js + 1) * 512], norm_ps[js][:p, :],
                    Act.Exp, scale=-1.0,
                )
        if variant == "full" and gi >= 1:
            _emit_obs_folds(gi - 1)
    if variant in ("core", "noex"):
        return
    _emit_obs_folds(2)
    for gi, (nm, sbase, col, p) in enumerate(GROUPS):
        sl = slice(gi * 512, (gi + 1) * 512)
        nc.sync.dma_start(out=obr[:p, sl], in_=obr_sb[:p, sl])


# ---------------- host side ----------------

def _col_octets(J):
    """a-octet list for column J under the K16 orientation."""
    octs = [2 * J, 2 * J + 1]
    for step in range(1, 8):
        I = (J + step) % NCOL
        octs += [2 * I, 2 * I + 1]
    if J >= 8:
        octs += [2 * (J - 8), 2 * (J - 8) + 1]  # antipodal in-edge
    return octs  # 16 or 18 octets


def _core_layout(d):
    """Per-core slot->a map and P_row credit mask."""
    colA, colB = d, d + 8
    octsA = _col_octets(colA)          # 16 octets -> slots 0..127
    octsB = _col_octets(colB)          # 18 octets -> slots 128..271
    slots = []
    for o in octsA + octsB:
        slots += list(range(o * 8, o * 8 + 8))
    slots = np.array(slots, np.int64)  # [272] a-index per slot
    credit = np.ones(NSLOT, bool)
    credit[0:16] = False               # col A self-octets
    credit[128:144] = False            # col B self-octets
    return slots, credit


def _prep_inputs(x, T):
    x = np.asarray(x, dtype=np.float32)
    T = np.asarray(T, dtype=np.float32)
    xT_bf = np.ascontiguousarray(x.T).astype(BF16)
    t2_bf = np.ascontiguousarray(
        T.reshape(IN_F, OUT_F, KD).transpose(0, 2, 1).reshape(IN_F, NO)
    ).astype(BF16)
    blk = np.zeros((128, 16 * 128), dtype=np.float32)
    for m in range(16):
        for a8 in range(8):
            for k in range(16):
                blk[a8 * 16 + k, m * 128 + m * 8 + a8] = 1.0
    blk_bf = blk.astype(BF16)
    in_maps = []
    for d in range(NCORES):
        slots, _ = _core_layout(d)
        bidx = np.concatenate([
            np.arange(d * CB, (d + 1) * CB),
            np.arange((d + 8) * CB, (d + 9) * CB),
        ])
        in_maps.append({
            "xT": np.ascontiguousarray(xT_bf[:, slots]),
            "t2": t2_bf,
            "xTb": np.ascontiguousarray(xT_bf[:, bidx]),
            "blk": blk_bf,
            "negI": (-np.eye(128, dtype=np.float32)).astype(BF16),
        })
    return in_maps


def _combine(x, results):
    o_b = np.zeros((B, OUT_F), np.float64)
    for d in range(NCORES):
        slots, credit = _core_layout(d)
        obc = np.asarray(results[d]["obc"], np.float64)   # [8, 512]
        obr = np.asarray(results[d]["obr"], np.float64)   # [128, 1536]
        # P_col: row r=col*4+js covers b 4js..4js+3 of that column
        for col, J in ((0, d), (1, d + 8)):
            pc = obc[col * 4:(col + 1) * 4].reshape(CB, OUT_F)
            o_b[J * CB:(J + 1) * CB] += pc
        # P_row: credited slots; obr group chunk [p, 512] = 4 b-folds x o
        for gi, (nm, sbase, col, p) in enumerate(GROUPS):
            rows = obr[:p, gi * 512:(gi + 1) * 512]
            rows = rows.reshape(p, 4, OUT_F).sum(axis=1)
            for i in range(p):
                s = sbase + i
                if credit[s]:
                    o_b[slots[s]] += rows[i]
    return np.concatenate(
        [np.asarray(x, np.float32), o_b.astype(np.float32)], axis=1
    )


_NC_CACHE = {}


def run(x, T, trace=False, **spmd_kwargs):
    if "nc" not in _NC_CACHE:
        _NC_CACHE["nc"] = _build_kernel()
    nc = _NC_CACHE["nc"]
    in_maps = _prep_inputs(x, T)
    res = run_bass_kernel_spmd(
        nc, in_maps, core_ids=list(range(NCORES)), trace=trace, **spmd_kwargs
    )
    out = _combine(x, res.results)
    return out, res


def kernel(x, T):
    out, _ = run(x, T, trace=False)
    return out

